# revision 6
# baseline (speedup 1.0000x reference)
"""ArcFace loss (B=8192, D=512, C=500000) on 8 TRN2 NeuronCores.

Strategy (classic partial-FC):
  - Host routes W rows into 8 disjoint shards of C/8 rows such that core k's
    shard contains the centers for batch rows [k*B/8, (k+1)*B/8) (labels are
    distinct).  Upload = exactly one copy of W across the 8 cores.
  - Each core: indirect-gather its 1024 centers from its local shard,
    L2-normalize rows (and its 1024 x rows), PE-transpose both, AllGather the
    bf16 transposed centers -> every core holds cnT for all 8192 columns.
  - Fused cos-matmul + log-sum-exp: for each [128 x 512] block, 4 bf16
    matmuls accumulate in PSUM, then one ScalarE Exp(scale=S) with accum_out
    reduces the row-wise sum of exp(S*cos) directly - the B x B matrix never
    exists in HBM.
  - The arcface margin only perturbs the diagonal, so it is applied as
    per-row scalar corrections: sumexp += exp(S*t') - exp(S*t), and the
    target-logit term uses t' = t*cos(M) - sqrt(1-t^2)*sin(M)  (exact
    identity for cos(arccos(t)+M)).
  - loss = (1/B) * sum_i [A1*lse_i - A2*t'_i]  with
    A1 = (1-eps) + eps*B/C, A2 = (1-eps)*S.  The (eps/C)*sum_j logits_ij
    term is dropped: |contribution| ~ 7e-6 absolute on a loss of ~39, below
    fp32 resolution of the result.
  - Per-core partial sums ([1,1] f32) are summed on host and divided by B.
"""

import sys

if "/opt/trn_rl_repo" not in sys.path:
    sys.path.insert(0, "/opt/trn_rl_repo")

import math
from contextlib import ExitStack

import numpy as np

import concourse.bacc as bacc
import concourse.bass as bass
import concourse.tile as tile
from concourse import mybir
from concourse.bass_utils import run_bass_kernel_spmd
from concourse.masks import make_identity

F32 = mybir.dt.float32
BF16 = mybir.dt.bfloat16
I32 = mybir.dt.int32
P = 128

# problem constants (hardcoded; kernel.py must be self-contained)
B, D, C = 8192, 512, 500000
NCORES = 8
MARGIN, S_SCALE, EPS = 0.5, 64.0, 0.1


def build_nc(b, d, csh, ncores, s_scale, margin, eps, c_total):
    """Build + compile the (identical-on-every-core) bass graph.

    b: global batch; d: feature dim; csh: W-shard rows per core.
    """
    bl = b // ncores          # local batch rows
    nt = bl // P              # local row tiles
    kc_n = d // P             # contraction chunks
    nb = min(512, bl)         # matmul moving free dim
    nh = bl // nb             # column half-blocks per rank block
    a1 = (1.0 - eps) + eps * b / c_total
    a2 = (1.0 - eps) * s_scale
    cos_m = float(math.cos(margin))
    sin_m = float(math.sin(margin))

    nc = bacc.Bacc(
        "TRN2",
        target_bir_lowering=False,
        debug=False,
        enable_asserts=False,
        num_devices=ncores,
    )
    w_ext = nc.dram_tensor("w", [csh, d], F32, kind="ExternalInput")
    x_ext = nc.dram_tensor("x", [bl, d], F32, kind="ExternalInput")
    idx_ext = nc.dram_tensor("idx", [P, nt], I32, kind="ExternalInput")
    out_ext = nc.dram_tensor("out", [1, 1], F32, kind="ExternalOutput")

    with tile.TileContext(nc) as tc:
        es = ExitStack()
        const = es.enter_context(tc.tile_pool(name="const", bufs=1))
        gath = es.enter_context(tc.tile_pool(name="gath", bufs=2))
        small = es.enter_context(tc.tile_pool(name="small", bufs=3))
        dram = es.enter_context(tc.tile_pool(name="dram", bufs=1, space="DRAM"))

        ident = const.tile([P, P], F32, name="ident")
        make_identity(nc, ident[:])

        idx_sb = const.tile([P, nt], I32, name="idx_sb")
        nc.sync.dma_start(out=idx_sb[:], in_=idx_ext[:, :])

        cn = [const.tile([P, d], F32, name=f"cn{t}") for t in range(nt)]
        xn = [const.tile([P, d], F32, name=f"xn{t}") for t in range(nt)]
        xnT = [const.tile([P, bl], BF16, name=f"xnT{k}") for k in range(kc_n)]
        cnTl = [const.tile([P, bl], BF16, name=f"cnTl{k}") for k in range(kc_n)]
        tcol = const.tile([P, nt], F32, name="tcol")
        se = [const.tile([P, nh * ncores], F32, name=f"se{m}") for m in range(nt)]

        cc_in = dram.tile([d, bl], BF16, name="cc_in")
        cc_out = dram.tile([ncores * d, bl], BF16, name="cc_out", addr_space="Shared")

        with tc.tile_pool(name="tp_psum", bufs=2, space="PSUM") as tp_psum:
            # --- center path: gather -> normalize -> transpose (feeds the AG) ---
            for t in range(nt):
                cent = gath.tile([P, d], F32, name="cent")
                nc.gpsimd.indirect_dma_start(
                    out=cent[:],
                    out_offset=None,
                    in_=w_ext[:, :],
                    in_offset=bass.IndirectOffsetOnAxis(ap=idx_sb[:, t : t + 1], axis=0),
                )
                sq = gath.tile([P, d], F32, name="sqc")
                ssq = small.tile([P, 1], F32, name="ssqc")
                nc.scalar.activation(
                    out=sq[:], in_=cent[:],
                    func=mybir.ActivationFunctionType.Square, accum_out=ssq[:],
                )
                nrm = small.tile([P, 1], F32, name="nrmc")
                nc.scalar.sqrt(nrm[:], ssq[:])
                rec = small.tile([P, 1], F32, name="recc")
                nc.vector.reciprocal(rec[:], nrm[:])
                nc.scalar.mul(out=cn[t][:], in_=cent[:], mul=rec[:])
                for kk in range(kc_n):
                    pt = tp_psum.tile([P, P], F32, name="ptc")
                    nc.tensor.transpose(
                        out=pt[:], in_=cn[t][:, kk * P : (kk + 1) * P], identity=ident[:]
                    )
                    nc.vector.tensor_copy(
                        out=cnTl[kk][:, t * P : (t + 1) * P], in_=pt[:]
                    )

            # stage transposed bf16 centers to DRAM and AllGather
            for kk in range(kc_n):
                nc.gpsimd.dma_start(out=cc_in[kk * P : (kk + 1) * P, :], in_=cnTl[kk][:])
            nc.gpsimd.collective_compute(
                "AllGather",
                mybir.AluOpType.bypass,
                replica_groups=[list(range(ncores))],
                ins=[cc_in[:].opt()],
                outs=[cc_out[:].opt()],
            )

            # --- x path (overlaps the collective) ---
            for t in range(nt):
                xt = gath.tile([P, d], F32, name="xt")
                nc.sync.dma_start(out=xt[:], in_=x_ext[t * P : (t + 1) * P, :])
                sqx = gath.tile([P, d], F32, name="sqx")
                ssqx = small.tile([P, 1], F32, name="ssqx")
                nc.scalar.activation(
                    out=sqx[:], in_=xt[:],
                    func=mybir.ActivationFunctionType.Square, accum_out=ssqx[:],
                )
                # reference clamps ||x|| at 1e-12; never binds for nonzero rows
                nrmx = small.tile([P, 1], F32, name="nrmx")
                nc.scalar.sqrt(nrmx[:], ssqx[:])
                recx = small.tile([P, 1], F32, name="recx")
                nc.vector.reciprocal(recx[:], nrmx[:])
                nc.scalar.mul(out=xn[t][:], in_=xt[:], mul=recx[:])
                for kk in range(kc_n):
                    ptx = tp_psum.tile([P, P], F32, name="ptx")
                    nc.tensor.transpose(
                        out=ptx[:], in_=xn[t][:, kk * P : (kk + 1) * P], identity=ident[:]
                    )
                    nc.vector.tensor_copy(
                        out=xnT[kk][:, t * P : (t + 1) * P], in_=ptx[:]
                    )
                # target cosine t_i = <xn_i, cn_i>
                dots = gath.tile([P, d], F32, name="dots")
                nc.vector.tensor_tensor(
                    out=dots[:], in0=xn[t][:], in1=cn[t][:],
                    op=mybir.AluOpType.mult,
                )
                nc.vector.tensor_reduce(
                    out=tcol[:, t : t + 1], in_=dots[:],
                    axis=mybir.AxisListType.X, op=mybir.AluOpType.add,
                )

        # --- margin terms on [P, nt] ---
        tsq = const.tile([P, nt], F32, name="tsq")
        nc.vector.tensor_tensor(
            out=tsq[:], in0=tcol[:], in1=tcol[:], op=mybir.AluOpType.mult
        )
        s1m = const.tile([P, nt], F32, name="s1m")
        nc.scalar.activation(
            out=s1m[:], in_=tsq[:], func=mybir.ActivationFunctionType.Sqrt,
            bias=1.0, scale=-1.0,
        )  # sqrt(1 - t^2)
        tpa = const.tile([P, nt], F32, name="tpa")
        nc.vector.tensor_scalar_mul(out=tpa[:], in0=tcol[:], scalar1=cos_m)
        tpb = const.tile([P, nt], F32, name="tpb")
        nc.vector.tensor_scalar_mul(out=tpb[:], in0=s1m[:], scalar1=sin_m)
        tpcol = const.tile([P, nt], F32, name="tpcol")
        nc.vector.tensor_tensor(
            out=tpcol[:], in0=tpa[:], in1=tpb[:], op=mybir.AluOpType.subtract
        )
        expt = const.tile([P, nt], F32, name="expt")
        nc.scalar.activation(
            out=expt[:], in_=tcol[:], func=mybir.ActivationFunctionType.Exp,
            scale=s_scale,
        )
        exptp = const.tile([P, nt], F32, name="exptp")
        nc.scalar.activation(
            out=exptp[:], in_=tpcol[:], func=mybir.ActivationFunctionType.Exp,
            scale=s_scale,
        )
        ecorr = const.tile([P, nt], F32, name="ecorr")
        nc.vector.tensor_tensor(
            out=ecorr[:], in0=exptp[:], in1=expt[:], op=mybir.AluOpType.subtract
        )

        # --- main loop: cos matmul + fused exp/row-sum ---
        with (
            tc.tile_pool(name="rhsp", bufs=2) as rhsp,
            tc.tile_pool(name="expp", bufs=3) as expp,
            tc.tile_pool(name="mm_psum", bufs=6, space="PSUM") as mm_psum,
        ):
            for j in range(ncores):
                rhs = []
                for kk in range(kc_n):
                    r = rhsp.tile([P, bl], BF16, name=f"rhs{kk}")
                    nc.sync.dma_start(
                        out=r[:],
                        in_=cc_out[j * d + kk * P : j * d + (kk + 1) * P, :],
                    )
                    rhs.append(r)
                for m in range(nt):
                    for h in range(nh):
                        ps = mm_psum.tile([P, nb], F32, name="mmblk")
                        for kk in range(kc_n):
                            nc.tensor.matmul(
                                out=ps[:],
                                lhsT=xnT[kk][:, m * P : (m + 1) * P],
                                rhs=rhs[kk][:, h * nb : (h + 1) * nb],
                                start=(kk == 0),
                                stop=(kk == kc_n - 1),
                            )
                        scr = expp.tile([P, nb], F32, name="expscr")
                        nc.scalar.activation(
                            out=scr[:], in_=ps[:],
                            func=mybir.ActivationFunctionType.Exp,
                            scale=s_scale,
                            accum_out=se[m][:, nh * j + h : nh * j + h + 1],
                        )

        # --- finale: corrected lse -> row terms -> scalar partial ---
        secor = const.tile([P, nt], F32, name="secor")
        for m in range(nt):
            nc.vector.tensor_reduce(
                out=secor[:, m : m + 1], in_=se[m][:],
                axis=mybir.AxisListType.X, op=mybir.AluOpType.add,
            )
        secor2 = const.tile([P, nt], F32, name="secor2")
        nc.vector.tensor_tensor(
            out=secor2[:], in0=secor[:], in1=ecorr[:], op=mybir.AluOpType.add
        )
        lse = const.tile([P, nt], F32, name="lse")
        nc.scalar.activation(
            out=lse[:], in_=secor2[:], func=mybir.ActivationFunctionType.Ln
        )
        ra = const.tile([P, nt], F32, name="ra")
        nc.vector.tensor_scalar_mul(out=ra[:], in0=lse[:], scalar1=a1)
        rb = const.tile([P, nt], F32, name="rb")
        nc.vector.tensor_scalar_mul(out=rb[:], in0=tpcol[:], scalar1=a2)
        rterm = const.tile([P, nt], F32, name="rterm")
        nc.vector.tensor_tensor(
            out=rterm[:], in0=ra[:], in1=rb[:], op=mybir.AluOpType.subtract
        )
        rsum = const.tile([P, 1], F32, name="rsum")
        nc.vector.tensor_reduce(
            out=rsum[:], in_=rterm[:], axis=mybir.AxisListType.X,
            op=mybir.AluOpType.add,
        )
        ones = const.tile([P, 1], F32, name="ones")
        nc.vector.memset(ones[:], 1.0)
        with tc.tile_pool(name="fin_psum", bufs=1, space="PSUM") as fin_psum:
            fin = fin_psum.tile([1, 1], F32, name="fin")
            nc.tensor.matmul(out=fin[:], lhsT=ones[:], rhs=rsum[:], start=True, stop=True)
            res = const.tile([1, 1], F32, name="res")
            nc.vector.tensor_copy(out=res[:], in_=fin[:])
        nc.sync.dma_start(out=out_ext[:, :], in_=res[:])

        es.close()

    nc.compile()
    return nc


def make_in_maps(x, labels, W, ncores=NCORES):
    """Host-side sharding: route W rows so core k's shard holds the centers
    for batch rows [k*bl, (k+1)*bl).  Returns per-core input dicts."""
    b, d = x.shape
    c = W.shape[0]
    bl = b // ncores
    csh = c // ncores
    nt = bl // P
    labels = np.asarray(labels).astype(np.int64)
    assert len(np.unique(labels)) == b, "routing assumes distinct labels"

    owner = np.full(c, -1, np.int8)
    for k in range(ncores):
        owner[labels[k * bl : (k + 1) * bl]] = k
    free_rows = np.flatnonzero(owner < 0)
    pos = 0
    in_maps = []
    for k in range(ncores):
        mine = np.flatnonzero(owner == k)
        need = csh - len(mine)
        extra = free_rows[pos : pos + need]
        pos += need
        rows = np.sort(np.concatenate([mine, extra]))
        lab = labels[k * bl : (k + 1) * bl]
        loc = np.searchsorted(rows, lab)
        assert np.array_equal(rows[loc], lab)
        idx = np.ascontiguousarray(loc.astype(np.int32).reshape(nt, P).T)
        in_maps.append(
            {
                "w": np.ascontiguousarray(W[rows]),
                "x": np.ascontiguousarray(x[k * bl : (k + 1) * bl]),
                "idx": idx,
            }
        )
    return in_maps


_compiled_nc = None


def get_compiled():
    global _compiled_nc
    if _compiled_nc is None:
        _compiled_nc = build_nc(
            B, D, C // NCORES, NCORES, S_SCALE, MARGIN, EPS, C
        )
    return _compiled_nc


def run(x, labels, W, trace=False, trace_cores=None):
    nc = get_compiled()
    in_maps = make_in_maps(
        np.asarray(x, dtype=np.float32), labels, np.asarray(W, dtype=np.float32)
    )
    res = run_bass_kernel_spmd(
        nc,
        in_maps,
        core_ids=list(range(NCORES)),
        trace=trace,
        trace_cores=trace_cores,
    )
    total = sum(float(r["out"][0, 0]) for r in res.results)
    return np.float32(total / B), res


def kernel(**inputs):
    loss, _ = run(inputs["x"], inputs["labels"], inputs["W"])
    return loss
